# revision 3
# baseline (speedup 1.0000x reference)
"""KoLeo loss kernel for 8 Trainium2 NeuronCores.

Computation (reference semantics):
    v = latents.squeeze()            # [N, D] f32, N=16384, D=64
    dp = v @ v.T ; dp[i,i] = -1      # NxN scores, diagonal excluded
    idx = argmax(dp, axis=1)         # nearest neighbor by dot product
    dist = ||v - v[idx] + 1e-6||_2
    out = mean(relu(-log(dist * N)))

Sharding: rows are block-sharded 2048/core.  Each core receives a copy of
v whose ROWS ARE ROTATED by -core*2048 so that the self-match diagonal of
its local [2048, 16384] score block always lands in column-chunk 0 at
offset (row % 2048) -- this keeps the SPMD program identical on all cores.

Per core pipeline:
  PE    : bf16 matmuls [K=64] -> PSUM chunks [128, 2048] f32
          (+ one -1e30*I accumulate matmul to suppress the diagonal)
  DVE   : ONE fused custom op per chunk: pack = round(x*16)*16384 + col,
          accumulated with MAX -> per-chunk packed (value, argmax col).
          The rounding uses the fp32 magic-constant trick so the packed
          value is exact; max over packed values == lexicographic
          (quantized value, column) max.
  tail  : unpack global argmax column, indirect-DMA gather v[idx],
          exact f32 distance, ln, clamp, DMA out per-row koleo values.
Host: mean of the 8x2048 per-row values.
"""

import math

import ml_dtypes
import numpy as np

N = 16384
D = 64
NCORES = 8
ROWS = N // NCORES  # 2048 rows per core
P = 128  # partitions
NT = ROWS // P  # 16 row-tiles per core
CHUNK = 2048  # columns scanned per custom-DVE op (4 PSUM banks)
NCHUNK = N // CHUNK  # 8
NEG_BIG = -1.0e30

# fp32 pack constants: pack = round(x*16)*16384 + col  (exact in fp32)
PACK_SCALE = 16.0 * 16384.0  # 262144.0
PACK_MAGIC = 3.0 * 2.0**22 * 16384.0  # rounds x*PACK_SCALE to multiples of 16384
UNPACK_MAGIC = 12582912.0  # 3 * 2^22: rounds to integers

_OP_NAME = "KOLEO_PACK_MAX_IDX"
_built = {}


def _register_pack_op():
    """Register the fused pack-max-idx custom DVE op (idempotent)."""
    from concourse import dve_ops
    from concourse.dve_spec import AluOp, C0, C1, C2, One, Spec, Src0, lower, scan
    from concourse.dve_uop import DveOpSpec

    if _OP_NAME in dve_ops._SUB_OPCODE_FOR_NAME:
        return next(op for op in dve_ops.OPS if op.name == _OP_NAME)

    def _reference(in0, in1, s0, s1, imm2):
        p = in0.shape[0]
        x = in0.astype(np.float32).reshape(p, -1)
        n = x.shape[1]
        z = (x * np.float32(s0) + np.float32(s1)) - np.float32(s1)
        col = (np.arange(n, dtype=np.float32) + np.float32(imm2))[None, :]
        body = (z + col).astype(np.float32)
        acc = body.max(axis=-1, keepdims=True)
        return body, acc

    # body[k] = round_16384(x*s0) + (imm2 + k); accum = max
    body = (Src0 * C0 + C1 - C1) + scan(AluOp.ADD, One, init=C2 - One)
    spec = Spec(body=body, accum=AluOp.MAX, reference=_reference)

    row = max(dve_ops._SUB_OPCODE_FOR_NAME.values()) + 1
    shas = {}
    for ver in ("v3", "v4"):
        uops = lower(spec, ver=ver)
        shas[ver] = DveOpSpec(
            name=_OP_NAME, opcode=row, uops=uops, rd1_en=False
        ).sha(ver)

    op = dve_ops.DveOp(_OP_NAME, spec, subdim=False, uops_sha=shas)
    dve_ops.OPS.append(op)
    dve_ops._SUB_OPCODE_FOR_NAME[_OP_NAME] = row
    dve_ops.CUSTOM_DVE_SPECS[_OP_NAME] = spec
    return op


def _build_nc():
    """Build + compile the per-core Bass program (same NEFF on all cores)."""
    if "nc" in _built:
        return _built["nc"]

    import concourse.bass as bass
    import concourse.mybir as mybir
    import concourse.tile as tile
    from concourse import bacc

    pack_op = _register_pack_op()

    f32 = mybir.dt.float32
    bf16 = mybir.dt.bfloat16
    i32 = mybir.dt.int32
    Alu = mybir.AluOpType

    nc = bacc.Bacc(None, target_bir_lowering=False)

    vt_rot = nc.declare_dram_parameter("vt_rot", [D, N], bf16, isOutput=False)
    vrows_t = nc.declare_dram_parameter("vrows_t", [D, ROWS], bf16, isOutput=False)
    vrows_sb = nc.declare_dram_parameter("vrows_sb", [P, NT, D], f32, isOutput=False)
    vrot = nc.declare_dram_parameter("vrot", [N, D], f32, isOutput=False)
    out_d = nc.declare_dram_parameter("out", [P, NT], f32, isOutput=True)

    neg_eye = nc.inline_tensor(
        (np.eye(P, dtype=np.float32) * NEG_BIG).astype(ml_dtypes.bfloat16), "neg_eye"
    )
    pos_eye = nc.inline_tensor(
        np.eye(P, dtype=np.float32).astype(ml_dtypes.bfloat16), "pos_eye"
    )

    with tile.TileContext(nc) as tc:
        with (
            tc.tile_pool(name="consts", bufs=1) as consts,
            tc.tile_pool(name="psum", bufs=2, space="PSUM") as psum_pool,
            tc.tile_pool(name="junk", bufs=2) as junk_pool,
            tc.tile_pool(name="small", bufs=1) as small,
        ):
            # ---- load inputs ----
            vt_sb = consts.tile([D, N], bf16)
            for c in range(NCHUNK):
                sl = slice(c * CHUNK, (c + 1) * CHUNK)
                nc.sync.dma_start(vt_sb[:, sl], vt_rot[:, sl])
            vrt_sb = consts.tile([D, ROWS], bf16)
            nc.sync.dma_start(vrt_sb[:], vrows_t[:])
            vr_sb = consts.tile([P, NT, D], f32)
            nc.sync.dma_start(vr_sb[:], vrows_sb[:])
            negI_sb = consts.tile([P, P], bf16)
            nc.sync.dma_start(negI_sb[:], neg_eye[:])
            posI_sb = consts.tile([P, P], bf16)
            nc.sync.dma_start(posI_sb[:], pos_eye[:])

            bm = small.tile([P, NT, NCHUNK], f32)  # packed per-chunk maxima

            # ---- main loop: matmuls + fused max/argmax scan ----
            for t in range(NT):
                lhsT = vrt_sb[:, t * P : (t + 1) * P]  # [64, 128] bf16
                diag_bank = t // 4  # 512-wide bank of chunk 0 with the diagonal
                for c in range(NCHUNK):
                    pt = psum_pool.tile([P, CHUNK], f32)
                    for m in range(4):
                        col0 = c * CHUNK + m * 512
                        is_diag = c == 0 and m == diag_bank
                        nc.tensor.matmul(
                            pt[:, m * 512 : (m + 1) * 512],
                            lhsT,
                            vt_sb[:, col0 : col0 + 512],
                            start=True,
                            stop=not is_diag,
                        )
                        if is_diag:
                            # accumulate -1e30 * I over the self-match block
                            nc.tensor.matmul(
                                pt[:, t * P : t * P + P],
                                negI_sb[:],
                                posI_sb[:],
                                start=False,
                                stop=True,
                            )
                    junk = junk_pool.tile([P, CHUNK], f32)
                    nc.vector._custom_dve(
                        pack_op,
                        out=junk[:],
                        in0=pt[:],
                        s0=PACK_SCALE,
                        s1=PACK_MAGIC,
                        imm2=float(c * CHUNK),
                        accum_out=bm[:, t, c : c + 1],
                    )

            # ---- global packed max per row, unpack argmax column ----
            g = small.tile([P, NT], f32)
            nc.vector.tensor_reduce(g[:], bm[:], axis=mybir.AxisListType.X, op=Alu.max)

            u1 = small.tile([P, NT], f32)
            nc.vector.tensor_scalar(
                out=u1[:], in0=g[:],
                scalar1=1.0 / 16384.0, scalar2=UNPACK_MAGIC,
                op0=Alu.mult, op1=Alu.add,
            )
            u2 = small.tile([P, NT], f32)  # qr in {q, q+1}
            nc.vector.tensor_scalar(
                out=u2[:], in0=u1[:], scalar1=UNPACK_MAGIC, scalar2=None,
                op0=Alu.subtract,
            )
            u3 = small.tile([P, NT], f32)  # -16384*qr
            nc.vector.tensor_scalar(
                out=u3[:], in0=u2[:], scalar1=-16384.0, scalar2=None, op0=Alu.mult
            )
            u4 = small.tile([P, NT], f32)  # pos' = col or col-16384
            nc.vector.tensor_tensor(out=u4[:], in0=g[:], in1=u3[:], op=Alu.add)
            u5 = small.tile([P, NT], f32)
            nc.vector.tensor_scalar(
                out=u5[:], in0=u4[:], scalar1=0.0, scalar2=None, op0=Alu.is_lt
            )
            u6 = small.tile([P, NT], f32)
            nc.vector.tensor_scalar(
                out=u6[:], in0=u5[:], scalar1=16384.0, scalar2=None, op0=Alu.mult
            )
            jcol = small.tile([P, NT], f32)
            nc.vector.tensor_tensor(out=jcol[:], in0=u4[:], in1=u6[:], op=Alu.add)
            jidx = small.tile([P, NT], i32)
            nc.vector.tensor_copy(jidx[:], jcol[:])

            # ---- gather nearest neighbors, exact f32 distance ----
            gat = small.tile([P, NT, D], f32)
            for t in range(NT):
                nc.gpsimd.indirect_dma_start(
                    out=gat[:, t, :],
                    out_offset=None,
                    in_=vrot[:],
                    in_offset=bass.IndirectOffsetOnAxis(
                        ap=jidx[:, t : t + 1], axis=0
                    ),
                )

            diff = small.tile([P, NT, D], f32)
            nc.vector.tensor_tensor(
                out=diff[:], in0=vr_sb[:], in1=gat[:], op=Alu.subtract
            )
            diff2 = small.tile([P, NT, D], f32)
            nc.vector.tensor_scalar(
                out=diff2[:], in0=diff[:], scalar1=1.0e-6, scalar2=None, op0=Alu.add
            )
            dsq = small.tile([P, NT, D], f32)
            nc.vector.tensor_tensor(out=dsq[:], in0=diff2[:], in1=diff2[:], op=Alu.mult)
            s2 = small.tile([P, NT], f32)
            nc.vector.tensor_reduce(s2[:], dsq[:], axis=mybir.AxisListType.X, op=Alu.add)

            lns = small.tile([P, NT], f32)
            nc.scalar.activation(lns[:], s2[:], mybir.ActivationFunctionType.Ln)
            kol = small.tile([P, NT], f32)
            nc.vector.tensor_scalar(
                out=kol[:], in0=lns[:],
                scalar1=-0.5, scalar2=-math.log(float(N)),
                op0=Alu.mult, op1=Alu.add,
            )
            if DEBUG_NO_CLAMP:
                nc.sync.dma_start(out_d[:], kol[:])
            else:
                kz = small.tile([P, NT], f32)
                nc.vector.tensor_scalar(
                    out=kz[:], in0=kol[:], scalar1=0.0, scalar2=None, op0=Alu.max
                )
                nc.sync.dma_start(out_d[:], kz[:])

    nc.compile()
    _built["nc"] = nc
    return nc


def _prep_in_maps(v: np.ndarray) -> list[dict]:
    in_maps = []
    for c in range(NCORES):
        vr = np.roll(v, -c * ROWS, axis=0)
        rows = v[c * ROWS : (c + 1) * ROWS]
        in_maps.append(
            {
                "vt_rot": np.ascontiguousarray(vr.T).astype(ml_dtypes.bfloat16),
                "vrows_t": np.ascontiguousarray(rows.T).astype(ml_dtypes.bfloat16),
                "vrows_sb": np.ascontiguousarray(
                    rows.reshape(NT, P, D).transpose(1, 0, 2)
                ),
                "vrot": np.ascontiguousarray(vr),
            }
        )
    return in_maps


# test.py can flip these to profile the run
TRACE = False
DEBUG_NO_CLAMP = False
LAST_RESULT = {}


def kernel(latents: np.ndarray) -> np.ndarray:
    from concourse.bass_utils import run_bass_kernel_spmd

    v = np.asarray(latents, dtype=np.float32).reshape(N, D)
    nc = _build_nc()
    in_maps = _prep_in_maps(v)

    kwargs = {}
    if TRACE:
        kwargs = dict(trace=True, stitch_traces=False)
    res = run_bass_kernel_spmd(nc, in_maps, core_ids=list(range(NCORES)), **kwargs)
    LAST_RESULT["res"] = res

    vals = np.concatenate([r["out"].reshape(-1) for r in res.results])
    return np.array(np.mean(vals), dtype=np.float32)


# revision 5
# speedup vs baseline: 1.2660x; 1.2660x over previous
"""KoLeo loss kernel for 8 Trainium2 NeuronCores.

Reference semantics:
    v = latents.squeeze()            # [N, D] f32, N=16384, D=64
    dp = v @ v.T ; dp[i,i] = -1      # NxN scores, diagonal excluded
    idx = argmax(dp, axis=1)         # nearest neighbor by dot product
    dist = ||v - v[idx] + 1e-6||_2
    out = mean(relu(-log(dist * N)))

Sharding: rows are block-sharded 2048/core.  Each core gets a copy of v
whose rows are ROTATED by -core*2048, so the self-match diagonal of its
local [2048, 16384] score block always lands at pair-column (row//2) --
the SPMD program is identical on all cores.

Pairwise-max trick: max(a, b) = (a + b + |a-b|) / 2.  The host ships
paired sums w = v[0::2]+v[1::2] and diffs u = v[0::2]-v[1::2]; the PE
computes dp-sums s = rows @ w.T and dp-diffs d = rows @ u.T (same FLOPs
as plain dp).  ScalarE takes |d| out of PSUM (its only job), and a
single fused custom VectorE op consumes (s from PSUM, |d| from SBUF) at
one output/cycle -- i.e. TWO dp elements per DVE cycle:

    pack = round_16384((s + |d|) * 8) + (pair_idx + 1);  accum = max

The fp32 magic-constant rounding makes the pack exact, so max over
packs == lexicographic (quantized pair-max, pair index) max.  The
diagonal is excluded exactly: accumulate -2^17 at the self position of
the SUM stream and -+2^17 (sign by parity) on the DIFF stream, which
turns the self-pair's max into its partner's value.

PE runs cold (1.2 GHz) on this part, so matmuls are row-packed with
tile_position: even row-tiles use array rows 0-63, odd tiles rows
64-127 (operands duplicated in SBUF partitions 64-127), two matmuls in
flight concurrently.

Tail: recover the winning pair, gather both members, pick the larger
exact f32 dot (partner forced if the winning pair is the self-pair),
exact f32 distance, ln, clamp, DMA out.  Host: mean of 8x2048 values.
"""

import math

import ml_dtypes
import numpy as np

N = 16384
D = 64
NCORES = 8
ROWS = N // NCORES  # 2048 rows per core
P = 128  # partitions
NT = ROWS // P  # 16 row-tiles per core
NPAIR = N // 2  # 8192 pair columns
CHUNK = 512  # pair columns per scan chunk (1 PSUM bank)
NCHUNK = NPAIR // CHUNK  # 16
BIG = 131072.0  # 2^17: diagonal suppression (exact in bf16, no f32 cancellation)

# fp32 pack constants: pack = round_16384((s+|d|)*8*16384) + pair_idx+1 (exact)
PACK_SCALE = 8.0 * 16384.0
PACK_MAGIC = 3.0 * 2.0**22 * 16384.0  # rounds to multiples of 16384
UNPACK_MAGIC = 12582912.0  # 3 * 2^22: rounds to integers

_OP_NAME = "KOLEO_PAIR_PACK_MAX"
_built = {}


def _register_pack_op():
    """Register the fused pair-max/argmax custom DVE op (idempotent)."""
    from concourse import dve_ops
    from concourse.dve_spec import (
        AluOp, C0, C1, One, Spec, Src0, Src1, Zero, lower, scan,
    )
    from concourse.dve_uop import DveOpSpec

    if _OP_NAME in dve_ops._SUB_OPCODE_FOR_NAME:
        return next(op for op in dve_ops.OPS if op.name == _OP_NAME)

    def _reference(in0, in1, s0, s1, imm2):
        p = in0.shape[0]
        s = in0.astype(np.float32).reshape(p, -1)
        a = in1.astype(np.float32).reshape(p, s.shape[1])
        z = ((s + a) * np.float32(s0) + np.float32(s1)) - np.float32(s1)
        col = (np.arange(s.shape[1], dtype=np.float32) + 1.0)[None, :]
        body = (z + col).astype(np.float32)
        acc = np.maximum(body.max(axis=-1, keepdims=True), 0.0)
        return body, acc

    body = ((Src0 + Src1) * C0 + C1 - C1) + scan(AluOp.ADD, One)
    spec = Spec(body=body, accum=AluOp.MAX, accum_init=Zero, reference=_reference)

    row = max(dve_ops._SUB_OPCODE_FOR_NAME.values()) + 1
    shas = {}
    for ver in ("v3", "v4"):
        uops = lower(spec, ver=ver)
        shas[ver] = DveOpSpec(
            name=_OP_NAME, opcode=row, uops=uops, rd1_en=True
        ).sha(ver)

    op = dve_ops.DveOp(_OP_NAME, spec, subdim=False, uops_sha=shas)
    dve_ops.OPS.append(op)
    dve_ops._SUB_OPCODE_FOR_NAME[_OP_NAME] = row
    dve_ops.CUSTOM_DVE_SPECS[_OP_NAME] = spec
    return op


def _build_nc():
    """Build + compile the per-core Bass program (same NEFF on all cores)."""
    if "nc" in _built:
        return _built["nc"]

    import concourse.bass as bass
    import concourse.mybir as mybir
    import concourse.tile as tile
    from concourse import bacc

    pack_op = _register_pack_op()

    f32 = mybir.dt.float32
    bf16 = mybir.dt.bfloat16
    i32 = mybir.dt.int32
    Alu = mybir.AluOpType
    Act = mybir.ActivationFunctionType

    nc = bacc.Bacc(None, target_bir_lowering=False)

    # w/u/vrows_t are duplicated into partitions 64-127 for row-packing
    wt_d = nc.declare_dram_parameter("wt", [P, NPAIR], bf16, isOutput=False)
    ut_d = nc.declare_dram_parameter("ut", [P, NPAIR], bf16, isOutput=False)
    vrows_t = nc.declare_dram_parameter("vrows_t", [P, ROWS], bf16, isOutput=False)
    vrows_sb = nc.declare_dram_parameter("vrows_sb", [P, NT, D], f32, isOutput=False)
    vrot = nc.declare_dram_parameter("vrot", [N, D], f32, isOutput=False)
    out_d = nc.declare_dram_parameter("out", [P, NT], f32, isOutput=True)

    neg_eye = nc.inline_tensor(
        (np.eye(P, dtype=np.float32) * -BIG).astype(ml_dtypes.bfloat16), "neg_eye"
    )
    sgn = np.where(np.arange(P) % 2 == 0, -BIG, BIG).astype(np.float32)
    alt_eye = nc.inline_tensor(
        (np.eye(P, dtype=np.float32) * sgn).astype(ml_dtypes.bfloat16), "alt_eye"
    )
    half_np = np.zeros((P, P // 2), dtype=np.float32)
    half_np[np.arange(P), np.arange(P) // 2] = 1.0
    half_eye = nc.inline_tensor(half_np.astype(ml_dtypes.bfloat16), "half_eye")
    iota_np = np.tile(np.arange(NCHUNK, dtype=np.float32), (P, NT))
    iota_c = nc.inline_tensor(iota_np, "iota_c")  # [P, NT*NCHUNK]
    # local row index r = t*128+p, its pair base 2*(r//2) and partner r^1
    rloc = (np.arange(NT)[None, :] * P + np.arange(P)[:, None]).astype(np.float32)
    selfj0_np = 2.0 * np.floor(rloc / 2.0)
    partner_np = rloc + np.where(rloc % 2 == 0, 1.0, -1.0)
    selfj0_c = nc.inline_tensor(selfj0_np.astype(np.float32), "selfj0")
    partner_c = nc.inline_tensor(partner_np.astype(np.float32), "partner")

    with tile.TileContext(nc) as tc:
        with (
            tc.tile_pool(name="consts", bufs=1) as consts,
            tc.tile_pool(name="psum", bufs=2, space="PSUM") as psum_pool,
            tc.tile_pool(name="absp", bufs=4) as absp,
            tc.tile_pool(name="junk", bufs=2) as junk_pool,
            tc.tile_pool(name="small", bufs=1) as small,
        ):
            # ---- load inputs ----
            wt_sb = consts.tile([P, NPAIR], bf16)
            ut_sb = consts.tile([P, NPAIR], bf16)
            for c in range(0, NCHUNK, 2):
                sl = slice(c * CHUNK, (c + 2) * CHUNK)
                nc.sync.dma_start(wt_sb[:, sl], wt_d[:, sl])
                nc.sync.dma_start(ut_sb[:, sl], ut_d[:, sl])
            vrt_sb = consts.tile([P, ROWS], bf16)
            nc.sync.dma_start(vrt_sb[:], vrows_t[:])
            vr_sb = consts.tile([P, NT, D], f32)
            nc.sync.dma_start(vr_sb[:], vrows_sb[:])
            negI_sb = consts.tile([P, P], bf16)
            nc.sync.dma_start(negI_sb[:], neg_eye[:])
            altI_sb = consts.tile([P, P], bf16)
            nc.sync.dma_start(altI_sb[:], alt_eye[:])
            halfI_sb = consts.tile([P, P // 2], bf16)
            nc.sync.dma_start(halfI_sb[:], half_eye[:])
            iota_sb = consts.tile([P, NT, NCHUNK], f32)
            nc.sync.dma_start(iota_sb[:], iota_c[:])
            selfj0_sb = consts.tile([P, NT], f32)
            nc.sync.dma_start(selfj0_sb[:], selfj0_c[:])
            partner_sb = consts.tile([P, NT], f32)
            nc.sync.dma_start(partner_sb[:], partner_c[:])

            bm = small.tile([P, NT, NCHUNK], f32)  # packed per-chunk maxima

            # ---- main loop: row-tile pairs (tA rows 0-63, tB rows 64-127) ----
            for s in range(NT // 2):
                tA, tB = 2 * s, 2 * s + 1
                lhsA = vrt_sb[0:64, tA * P : (tA + 1) * P]
                lhsB = vrt_sb[64:128, tB * P : (tB + 1) * P]
                for c in range(NCHUNK):
                    sl = slice(c * CHUNK, (c + 1) * CHUNK)
                    psA = psum_pool.tile([P, CHUNK], f32)
                    pdA = psum_pool.tile([P, CHUNK], f32)
                    psB = psum_pool.tile([P, CHUNK], f32)
                    pdB = psum_pool.tile([P, CHUNK], f32)
                    dA = c == tA // 8  # this chunk holds tile A's self pairs
                    dB = c == tB // 8
                    offA = (tA % 8) * 64
                    offB = (tB % 8) * 64
                    nc.tensor.matmul(
                        psA[:], lhsA, wt_sb[0:64, sl], start=True, stop=not dA
                    )
                    nc.tensor.matmul(
                        psB[:], lhsB, wt_sb[64:128, sl], start=True, stop=not dB
                    )
                    nc.tensor.matmul(
                        pdA[:], lhsA, ut_sb[0:64, sl], start=True, stop=not dA
                    )
                    nc.tensor.matmul(
                        pdB[:], lhsB, ut_sb[64:128, sl], start=True, stop=not dB
                    )
                    if dA:
                        nc.tensor.matmul(
                            psA[:, offA : offA + 64], negI_sb[:], halfI_sb[:],
                            start=False, stop=True,
                        )
                        nc.tensor.matmul(
                            pdA[:, offA : offA + 64], altI_sb[:], halfI_sb[:],
                            start=False, stop=True,
                        )
                    if dB:
                        nc.tensor.matmul(
                            psB[:, offB : offB + 64], negI_sb[:], halfI_sb[:],
                            start=False, stop=True,
                        )
                        nc.tensor.matmul(
                            pdB[:, offB : offB + 64], altI_sb[:], halfI_sb[:],
                            start=False, stop=True,
                        )
                    adA = absp.tile([P, CHUNK], f32)
                    nc.scalar.activation(adA[:], pdA[:], Act.Abs)
                    junkA = junk_pool.tile([P, CHUNK], f32)
                    nc.vector._custom_dve(
                        pack_op, out=junkA[:], in0=psA[:], in1=adA[:],
                        s0=PACK_SCALE, s1=PACK_MAGIC,
                        accum_out=bm[:, tA, c : c + 1],
                    )
                    adB = absp.tile([P, CHUNK], f32)
                    nc.scalar.activation(adB[:], pdB[:], Act.Abs)
                    junkB = junk_pool.tile([P, CHUNK], f32)
                    nc.vector._custom_dve(
                        pack_op, out=junkB[:], in0=psB[:], in1=adB[:],
                        s0=PACK_SCALE, s1=PACK_MAGIC,
                        accum_out=bm[:, tB, c : c + 1],
                    )

            # ---- winner per row: chunk + local pair index ----
            g = small.tile([P, NT], f32)
            nc.vector.tensor_reduce(g[:], bm[:], axis=mybir.AxisListType.X, op=Alu.max)

            eqm = small.tile([P, NT, NCHUNK], f32)
            for t in range(NT):
                nc.vector.tensor_scalar(
                    out=eqm[:, t, :], in0=bm[:, t, :],
                    scalar1=g[:, t : t + 1], scalar2=None, op0=Alu.is_ge,
                )
            eqi = small.tile([P, NT, NCHUNK], f32)
            nc.vector.tensor_tensor(out=eqi[:], in0=eqm[:], in1=iota_sb[:], op=Alu.mult)
            cstar = small.tile([P, NT], f32)
            nc.vector.tensor_reduce(
                cstar[:], eqi[:], axis=mybir.AxisListType.X, op=Alu.max
            )

            # unpack local pair index (+1) from g (idx <= 512 so no round-up)
            u1 = small.tile([P, NT], f32)
            nc.vector.tensor_scalar(
                out=u1[:], in0=g[:],
                scalar1=1.0 / 16384.0, scalar2=UNPACK_MAGIC,
                op0=Alu.mult, op1=Alu.add,
            )
            u2 = small.tile([P, NT], f32)
            nc.vector.tensor_scalar(
                out=u2[:], in0=u1[:],
                scalar1=UNPACK_MAGIC, scalar2=-16384.0,
                op0=Alu.subtract, op1=Alu.mult,
            )
            idx1 = small.tile([P, NT], f32)
            nc.vector.tensor_tensor(out=idx1[:], in0=g[:], in1=u2[:], op=Alu.add)

            # j0 = 2*(cstar*CHUNK + idx1 - 1); j1 = j0 + 1
            ct = small.tile([P, NT], f32)
            nc.vector.tensor_scalar(
                out=ct[:], in0=cstar[:], scalar1=float(CHUNK), scalar2=None,
                op0=Alu.mult,
            )
            pairf = small.tile([P, NT], f32)
            nc.vector.tensor_tensor(out=pairf[:], in0=ct[:], in1=idx1[:], op=Alu.add)
            j0f = small.tile([P, NT], f32)
            nc.vector.tensor_scalar(
                out=j0f[:], in0=pairf[:], scalar1=2.0, scalar2=-2.0,
                op0=Alu.mult, op1=Alu.add,
            )
            j1f = small.tile([P, NT], f32)
            nc.vector.tensor_scalar(
                out=j1f[:], in0=j0f[:], scalar1=1.0, scalar2=None, op0=Alu.add
            )
            j0i = small.tile([P, NT], i32)
            nc.vector.tensor_copy(j0i[:], j0f[:])
            j1i = small.tile([P, NT], i32)
            nc.vector.tensor_copy(j1i[:], j1f[:])

            # ---- gather both pair members, pick larger exact dot ----
            gat0 = small.tile([P, NT, D], f32)
            gat1 = small.tile([P, NT, D], f32)
            for t in range(NT):
                nc.gpsimd.indirect_dma_start(
                    out=gat0[:, t, :], out_offset=None, in_=vrot[:],
                    in_offset=bass.IndirectOffsetOnAxis(ap=j0i[:, t : t + 1], axis=0),
                )
                nc.gpsimd.indirect_dma_start(
                    out=gat1[:, t, :], out_offset=None, in_=vrot[:],
                    in_offset=bass.IndirectOffsetOnAxis(ap=j1i[:, t : t + 1], axis=0),
                )

            pr0 = small.tile([P, NT, D], f32)
            nc.vector.tensor_tensor(out=pr0[:], in0=vr_sb[:], in1=gat0[:], op=Alu.mult)
            dot0 = small.tile([P, NT], f32)
            nc.vector.tensor_reduce(
                dot0[:], pr0[:], axis=mybir.AxisListType.X, op=Alu.add
            )
            pr1 = small.tile([P, NT, D], f32)
            nc.vector.tensor_tensor(out=pr1[:], in0=vr_sb[:], in1=gat1[:], op=Alu.mult)
            dot1 = small.tile([P, NT], f32)
            nc.vector.tensor_reduce(
                dot1[:], pr1[:], axis=mybir.AxisListType.X, op=Alu.add
            )
            sel1 = small.tile([P, NT], f32)  # 1.0 if dot1 > dot0
            nc.vector.tensor_tensor(out=sel1[:], in0=dot1[:], in1=dot0[:], op=Alu.is_gt)
            jsel = small.tile([P, NT], f32)
            nc.vector.tensor_tensor(out=jsel[:], in0=j0f[:], in1=sel1[:], op=Alu.add)

            # if the winning pair is the self-pair, force the partner
            meq = small.tile([P, NT], f32)  # 1.0 if j0 == 2*(r//2)
            nc.vector.tensor_tensor(
                out=meq[:], in0=j0f[:], in1=selfj0_sb[:], op=Alu.is_equal
            )
            dpar = small.tile([P, NT], f32)
            nc.vector.tensor_tensor(
                out=dpar[:], in0=partner_sb[:], in1=jsel[:], op=Alu.subtract
            )
            dfix = small.tile([P, NT], f32)
            nc.vector.tensor_tensor(out=dfix[:], in0=dpar[:], in1=meq[:], op=Alu.mult)
            jff = small.tile([P, NT], f32)
            nc.vector.tensor_tensor(out=jff[:], in0=jsel[:], in1=dfix[:], op=Alu.add)
            jfi = small.tile([P, NT], i32)
            nc.vector.tensor_copy(jfi[:], jff[:])

            gatf = small.tile([P, NT, D], f32)
            for t in range(NT):
                nc.gpsimd.indirect_dma_start(
                    out=gatf[:, t, :], out_offset=None, in_=vrot[:],
                    in_offset=bass.IndirectOffsetOnAxis(ap=jfi[:, t : t + 1], axis=0),
                )

            # ---- exact f32 distance, koleo, clamp ----
            diff = small.tile([P, NT, D], f32)
            nc.vector.tensor_tensor(
                out=diff[:], in0=vr_sb[:], in1=gatf[:], op=Alu.subtract
            )
            diff2 = small.tile([P, NT, D], f32)
            nc.vector.tensor_scalar(
                out=diff2[:], in0=diff[:], scalar1=1.0e-6, scalar2=None, op0=Alu.add
            )
            dsq = small.tile([P, NT, D], f32)
            nc.vector.tensor_tensor(out=dsq[:], in0=diff2[:], in1=diff2[:], op=Alu.mult)
            s2 = small.tile([P, NT], f32)
            nc.vector.tensor_reduce(s2[:], dsq[:], axis=mybir.AxisListType.X, op=Alu.add)

            lns = small.tile([P, NT], f32)
            nc.scalar.activation(lns[:], s2[:], Act.Ln)
            kol = small.tile([P, NT], f32)
            nc.vector.tensor_scalar(
                out=kol[:], in0=lns[:],
                scalar1=-0.5, scalar2=-math.log(float(N)),
                op0=Alu.mult, op1=Alu.add,
            )
            if DEBUG_NO_CLAMP:
                nc.sync.dma_start(out_d[:], kol[:])
            else:
                kz = small.tile([P, NT], f32)
                nc.vector.tensor_scalar(
                    out=kz[:], in0=kol[:], scalar1=0.0, scalar2=None, op0=Alu.max
                )
                nc.sync.dma_start(out_d[:], kz[:])

    nc.compile()
    _built["nc"] = nc
    return nc


def _prep_in_maps(v: np.ndarray) -> list[dict]:
    bf = ml_dtypes.bfloat16
    in_maps = []
    for c in range(NCORES):
        vr = np.roll(v, -c * ROWS, axis=0)
        w = vr[0::2] + vr[1::2]  # [NPAIR, D] f32
        u = vr[0::2] - vr[1::2]
        rows = v[c * ROWS : (c + 1) * ROWS]
        wt = np.ascontiguousarray(w.T).astype(bf)
        ut = np.ascontiguousarray(u.T).astype(bf)
        rt = np.ascontiguousarray(rows.T).astype(bf)
        in_maps.append(
            {
                "wt": np.concatenate([wt, wt], axis=0),
                "ut": np.concatenate([ut, ut], axis=0),
                "vrows_t": np.concatenate([rt, rt], axis=0),
                "vrows_sb": np.ascontiguousarray(
                    rows.reshape(NT, P, D).transpose(1, 0, 2)
                ),
                "vrot": np.ascontiguousarray(vr),
            }
        )
    return in_maps


# test.py can flip these to profile the run
TRACE = False
DEBUG_NO_CLAMP = False
LAST_RESULT = {}


def kernel(latents: np.ndarray) -> np.ndarray:
    from concourse.bass_utils import run_bass_kernel_spmd

    v = np.asarray(latents, dtype=np.float32).reshape(N, D)
    nc = _build_nc()
    in_maps = _prep_in_maps(v)

    kwargs = {}
    if TRACE:
        kwargs = dict(trace=True, stitch_traces=False)
    res = run_bass_kernel_spmd(nc, in_maps, core_ids=list(range(NCORES)), **kwargs)
    LAST_RESULT["res"] = res

    vals = np.concatenate([r["out"].reshape(-1) for r in res.results])
    return np.array(np.mean(vals), dtype=np.float32)
